# revision 41
# baseline (speedup 1.0000x reference)
"""AttentiveStatisticsPooling Trainium2 Bass kernel (v2).

Self-contained: builds + compiles + runs an 8-core SPMD Bass program.

Math (faithful to the reference module, including its x - mean**2 quirk):
  T_n     = #{l : l < lengths[n]*L}                     (exact fp32 compare)
  mean_g  = sum_{l<T} x / T                             [N, C]
  std_g   = sqrt(clamp(mean_g - mean_g^2, EPS))         (weights sum to 1 =>
                                                         the var-like term collapses)
  h       = tanh(s * relu(W1a@x + c) + t)               s,t = folded BN affine
            c = W1b@mean_g + W1c@std_g + b1             per-sample vector [A]
  e       = exp(W2@h)   (b2 dropped: softmax-invariant; zero anyway)
  mean    = sum_{l<T} e*x / sum_{l<T} e                 [N, C]
  std     = sqrt(clamp(mean - mean^2, EPS))
  out     = concat(mean, std)[:, :, None]               [N, 2C, 1]

No mask tensor on device: x tails are pre-zeroed on host, so sp = sum e*x is
unaffected by the tail (e_tail * 0 = 0). Only the softmax denominator needs
fixing: every tail column contributes the SAME e_tail = exp(W2@tanh(s*relu(c)+t))
(x=0 there), so se = se_raw - (W + 1 - T) * e_tail, where the +1 covers one
extra "tail column" appended to the last quad (used to compute e_tail itself
for free inside the wide exp op).

Sharding: data-parallel over N; 16 samples -> 8 cores x 2 slots, sorted by T
(slot 0 = 8 longest). One SPMD program, static slot widths = max T per slot.

Pipeline (per body == one full sample-pair iteration, unrolled x2 in the
timing loop so loads/stats of iteration k+1 hide under compute of k):
  loads(k+1) -> slot0: W1a->relu->tanh->[per cc,q: W2->exp(+se accum)->prod]
  -> slot1 same -> sx/stats(k+1) -> finals(k) -> out dma.
"""

import numpy as np
import ml_dtypes

N, C, L, A = 16, 512, 3000, 128
NCORES = 8
CC = C // 128          # 4 channel chunks of 128 partitions
BLK = 512              # psum bank width (fp32)
QUAD = 1024            # exp/prod op width (2 psum banks)
SXG = 2048             # sum-x group width
EPS = 1e-12
RSQRT_MAGIC = float(0x5F3759DF)
UNROLL = 32

BF16 = ml_dtypes.bfloat16

OPTS = {
    "se_on_act": True,     # exp accum_out on Act engine (else DVE sum pass)
    "skip_group_check": True,
    "unroll": UNROLL,
    "prod_pool_num": 0,    # of every prod_pool_den prod multiplies -> Pool
    "prod_pool_den": 5,
    "epool_bufs": 4,
    "act_sqrt": False,
    "newton_iters": 1,
    "w2wide": False,
    "spool_bufs": 2,
    "stagger": False,
    "dma2rings": True,
}


# ---------------------------------------------------------------- host prep

def _lengths_to_T(lengths):
    """Exact replica of the reference fp32 mask comparison."""
    idx = np.arange(L, dtype=np.float32)
    thresh = (lengths.astype(np.float32) * np.float32(L)).astype(np.float32)
    return (idx[None, :] < thresh[:, None]).sum(axis=1).astype(np.int64)


def _host_prep(x, lengths, W1, b1, bn_gamma, bn_beta, bn_mean, bn_var, W2, b2):
    x = np.asarray(x)
    Ts = np.maximum(_lengths_to_T(np.asarray(lengths)), 1)
    order = np.argsort(-Ts, kind="stable")
    slots = [order[:NCORES], order[NCORES:]]
    widths = [int(Ts[s].max()) for s in slots]

    def chunk_cols(m):  # [C, A] -> [128, CC*A], chunk cc at cols [cc*A:(cc+1)*A]
        return np.ascontiguousarray(
            m.reshape(CC, 128, m.shape[1]).transpose(1, 0, 2).reshape(128, -1))

    s = (np.asarray(bn_gamma) / np.sqrt(np.asarray(bn_var) + 1e-5)).astype(np.float32)
    t = (np.asarray(bn_beta) - np.asarray(bn_mean) * s).astype(np.float32)
    W1 = np.asarray(W1, dtype=np.float32)
    W2 = np.asarray(W2, dtype=np.float32)

    shared = {
        "w1aT": chunk_cols(np.ascontiguousarray(W1[:, :C].T)).astype(BF16),
        "w2T":  np.ascontiguousarray(W2.T).astype(BF16),           # [A, C]
        "w1bT": chunk_cols(np.ascontiguousarray(W1[:, C:2 * C].T)).astype(np.float32),
        "w1cT": chunk_cols(np.ascontiguousarray(W1[:, 2 * C:].T)).astype(np.float32),
        "svec": s.reshape(A, 1),
        "tvec": t.reshape(A, 1),
        "b1v":  np.asarray(b1, dtype=np.float32).reshape(A, 1),
    }

    in_maps, metas = [], []
    for core in range(NCORES):
        m = dict(shared)
        meta = []
        invT = np.zeros((1, 2), np.float32)
        wtail = np.zeros((1, 2), np.float32)
        for sl in range(2):
            n = int(slots[sl][core])
            T = int(Ts[n])
            W = widths[sl]
            xb = x[n, :, :W].astype(BF16)
            if T < W:
                xb[:, T:] = BF16(0)
            m[f"x{sl}"] = xb
            invT[0, sl] = 1.0 / T
            wtail[0, sl] = float(W + 1 - T)
            meta.append((n, T))
        m["invT"] = np.broadcast_to(invT, (128, 2)).copy()
        m["wtail"] = np.broadcast_to(wtail, (128, 2)).copy()
        in_maps.append(m)
        metas.append(meta)
    return in_maps, metas, widths


# ---------------------------------------------------------------- program

def _build_program(widths, loop=False):
    import concourse.bass as bass  # noqa: F401
    import concourse.tile as tile
    from concourse import bacc, mybir
    from contextlib import ExitStack

    f32, bf16, i32 = mybir.dt.float32, mybir.dt.bfloat16, mybir.dt.int32
    Alu = mybir.AluOpType
    Act = mybir.ActivationFunctionType
    sgc = OPTS["skip_group_check"]

    # per-slot static geometry
    W0MAX = max(widths)
    NB = [(w + BLK - 1) // BLK for w in widths]          # 512-blocks
    NQ = [(w + QUAD - 1) // QUAD for w in widths]        # 1024-quads
    NG = [(w + SXG - 1) // SXG for w in widths]          # sx groups
    for sl in range(2):
        wq_last = widths[sl] - QUAD * (NQ[sl] - 1)
        assert wq_last < QUAD, "tail-column trick needs wq_last < QUAD"

    nc = bacc.Bacc("TRN2", target_bir_lowering=False, debug=False,
                   num_devices=NCORES)
    reps = (nc.dram_tensor("reps", [1, 1], i32, kind="ExternalInput").ap()
            if loop else None)

    xs = [nc.dram_tensor(f"x{sl}", [C, widths[sl]], bf16,
                         kind="ExternalInput").ap() for sl in range(2)]
    invT = nc.dram_tensor("invT", [128, 2], f32, kind="ExternalInput").ap()
    wtail = nc.dram_tensor("wtail", [128, 2], f32, kind="ExternalInput").ap()
    w1aT = nc.dram_tensor("w1aT", [128, CC * A], bf16, kind="ExternalInput").ap()
    w2T = nc.dram_tensor("w2T", [A, C], bf16, kind="ExternalInput").ap()
    w1bT = nc.dram_tensor("w1bT", [128, CC * A], f32, kind="ExternalInput").ap()
    w1cT = nc.dram_tensor("w1cT", [128, CC * A], f32, kind="ExternalInput").ap()
    svec = nc.dram_tensor("svec", [A, 1], f32, kind="ExternalInput").ap()
    tvec = nc.dram_tensor("tvec", [A, 1], f32, kind="ExternalInput").ap()
    b1v = nc.dram_tensor("b1v", [A, 1], f32, kind="ExternalInput").ap()
    out = nc.dram_tensor("out", [128, 16], f32, kind="ExternalOutput").ap()

    with tile.TileContext(nc) as tc, ExitStack() as ctx:
        consts = ctx.enter_context(tc.tile_pool(name="consts", bufs=1))
        xpool = ctx.enter_context(tc.tile_pool(name="xpool", bufs=2))
        upool = ctx.enter_context(tc.tile_pool(name="upool", bufs=2))
        epool = ctx.enter_context(tc.tile_pool(name="epool",
                                               bufs=OPTS["epool_bufs"]))
        spool = ctx.enter_context(tc.tile_pool(name="spool",
                                               bufs=OPTS["spool_bufs"]))
        accp = ctx.enter_context(tc.tile_pool(name="accp", bufs=2))
        smalls = ctx.enter_context(tc.tile_pool(name="smalls", bufs=2))
        outp = ctx.enter_context(tc.tile_pool(name="outp", bufs=2))
        ph = ctx.enter_context(tc.tile_pool(name="ph", bufs=2, space="PSUM"))
        pa = ctx.enter_context(tc.tile_pool(name="pa", bufs=2, space="PSUM"))
        pc = ctx.enter_context(tc.tile_pool(name="pc", bufs=1, space="PSUM"))

        def load_const(ap_in, shape, dt, name):
            t_ = consts.tile(shape, dt, name=name, tag=name)
            nc.sync.dma_start(t_[:], ap_in)
            return t_

        w1aT_sb = load_const(w1aT, [128, CC * A], bf16, "w1aT_sb")
        w2T_sb = load_const(w2T, [A, C], bf16, "w2T_sb")
        w1bT_sb = load_const(w1bT, [128, CC * A], f32, "w1bT_sb")
        w1cT_sb = load_const(w1cT, [128, CC * A], f32, "w1cT_sb")
        svec_sb = load_const(svec, [A, 1], f32, "svec_sb")
        tvec_sb = load_const(tvec, [A, 1], f32, "tvec_sb")
        b1_sb = load_const(b1v, [A, 1], f32, "b1_sb")
        invT_sb = load_const(invT, [128, 2], f32, "invT_sb")
        wtail_sb = load_const(wtail, [128, 2], f32, "wtail_sb")
        z0 = consts.tile([A, 1], f32, name="z0", tag="z0")
        nc.vector.memset(z0[:], 0.0)

        def newton_sqrt(pool, var_t, w, iters, out=None):
            """Elementwise sqrt of a [128, w] fp32 tile (values >= EPS)."""
            yb = pool.tile([128, w], i32, tag="nt_yb")
            nc.vector.tensor_scalar(
                out=yb[:], in0=var_t[:].bitcast(i32), scalar1=-0.5,
                scalar2=RSQRT_MAGIC, op0=Alu.mult, op1=Alu.add)
            y = yb[:].bitcast(f32)
            for _ in range(iters):
                t1 = pool.tile([128, w], f32, tag="nt_t1")
                nc.vector.tensor_tensor(out=t1[:], in0=y, in1=y, op=Alu.mult)
                nc.vector.tensor_tensor(out=t1[:], in0=t1[:], in1=var_t[:],
                                        op=Alu.mult)
                nc.vector.tensor_scalar(
                    out=t1[:], in0=t1[:], scalar1=-0.5, scalar2=1.5,
                    op0=Alu.mult, op1=Alu.add)
                yn = pool.tile([128, w], f32, tag="nt_yn")
                nc.vector.tensor_tensor(out=yn[:], in0=y, in1=t1[:],
                                        op=Alu.mult)
                y = yn[:]
            if out is None:
                r = pool.tile([128, w], f32, tag="nt_r")
                out = r[:]
            nc.vector.tensor_tensor(out=out, in0=var_t[:], in1=y, op=Alu.mult)
            return out

        # -------------------------------------------------- pipeline stages

        def stage_load(k):
            """DMA both slots' x (merged [128, CC, W] layout, 2 DMAs per
            slot); returns per-slot lists of per-cc [128, W] views."""
            xt = []
            for sl in range(2):
                x3 = xpool.tile([128, CC, widths[sl]], bf16,
                                tag=f"x{sl}", name=f"x{sl}_{k}")
                xin = xs[sl].rearrange("(cc p) w -> p cc w", p=128)
                eng2 = nc.scalar if OPTS.get("dma2rings") else nc.sync
                nc.sync.dma_start(x3[:, 0:2], xin[:, 0:2, :])
                eng2.dma_start(x3[:, 2:4], xin[:, 2:4, :])
                xt.append([x3[:, cc] for cc in range(CC)])
            return xt

        def stats_sx_pieces(sl, xt, k):
            """Per-cc sum-x closures (one 4x op per cc); returns
            (pieces, xs2) with xs2[128, CC] holding the per-cc sums."""
            W = widths[sl]
            xs2 = accp.tile([128, CC], f32, tag=f"xs2_{sl}",
                            name=f"xs2_{sl}_{k}")

            def make_cc(cc):
                def go():
                    scr = spool.tile([128, W0MAX], bf16, tag="scr",
                                     name=f"sx{sl}_{cc}_{k}")
                    nc.vector.tensor_scalar(
                        out=scr[:, :W], in0=xt[sl][cc][:, :W],
                        scalar1=0.0, scalar2=None, op0=Alu.bypass,
                        op1=Alu.add, accum_out=xs2[:, cc:cc + 1])
                return go

            return [make_cc(cc) for cc in range(CC)], xs2

        def stats_finish(sl, xs3, k):
            """mean/std -> cvec for slot sl. Returns (cvec, fill_fn): the
            cvec tile exists immediately; fill_fn emits the ops (so it can be
            interleaved into an attention stream without head-of-line
            blocking the PE/DVE queues)."""
            cvec = smalls.tile([A, 1], f32, tag="cvec", name=f"cvec{sl}_{k}")

            def fill():
                self_fill(sl, xs3, k, cvec)
            return cvec, fill

        def self_fill(sl, xs2, k, cvec):
            mg = smalls.tile([128, CC], f32, tag="mg", name=f"mg{sl}_{k}")
            nc.vector.tensor_scalar(
                out=mg[:], in0=xs2[:], scalar1=invT_sb[:, sl:sl + 1],
                scalar2=None, op0=Alu.mult)
            vg = smalls.tile([128, CC], f32, tag="vg", name=f"vg{sl}_{k}")
            nc.vector.tensor_tensor(out=vg[:], in0=mg[:], in1=mg[:],
                                    op=Alu.mult)
            nc.vector.tensor_tensor(out=vg[:], in0=mg[:], in1=vg[:],
                                    op=Alu.subtract)
            nc.vector.tensor_scalar(out=vg[:], in0=vg[:], scalar1=EPS,
                                    scalar2=None, op0=Alu.max)
            if OPTS["act_sqrt"]:
                sg = smalls.tile([128, CC], f32, tag="sg", name=f"sg{sl}_{k}")
                nc.scalar.sqrt(out=sg[:], in_=vg[:])
                sg = sg[:]
            else:
                sg = newton_sqrt(smalls, vg, CC, OPTS["newton_iters"])
            c_ps = pc.tile([A, 1], f32, tag="c_ps", name=f"c_ps{sl}_{k}")
            for cc in range(CC):
                nc.tensor.matmul(
                    c_ps[:], w1bT_sb[:, cc * A:(cc + 1) * A], mg[:, cc:cc + 1],
                    start=(cc == 0), stop=False)
            for cc in range(CC):
                nc.tensor.matmul(
                    c_ps[:], w1cT_sb[:, cc * A:(cc + 1) * A], sg[:, cc:cc + 1],
                    start=False, stop=(cc == CC - 1))
            nc.vector.tensor_scalar(out=cvec[:], in0=c_ps[:],
                                    scalar1=b1_sb[:, 0:1], scalar2=None,
                                    op0=Alu.add)

        def stage_h_pieces(sl, xt, cvec, k):
            """W1a+relu for the slot, as a list of closures (one per block
            pair) for interleaving into another slot's attention stream;
            plus a tanh closure. Returns (pieces, tanh_fn, hfin)."""
            W, nb = widths[sl], NB[sl]
            u = upool.tile([A, widths[sl] + 1], bf16, tag=f"u{sl}",
                           name=f"u{sl}_{k}")
            hfin = upool.tile([A, widths[sl] + 1], bf16, tag=f"hf{sl}",
                              name=f"hf{sl}_{k}")

            npair = (nb + 1) // 2

            def make_piece(p, cc):
                # one W1a chunk over a block pair (~0.4us PE); the last chunk
                # of a pair also emits the pair's relus (DVE)
                def go():
                    bs = [b for b in (2 * p, 2 * p + 1) if b < nb]
                    if cc == 0:
                        pair_hps[p] = {
                            b: ph.tile([A, BLK], f32, tag="h_ps",
                                       name=f"h{sl}_{b}_{k}") for b in bs}
                    hps = pair_hps[p]
                    for b in bs:
                        wb = min(BLK, W - b * BLK)
                        nc.tensor.matmul(
                            hps[b][:, :wb],
                            w1aT_sb[:, cc * A:(cc + 1) * A],
                            xt[sl][cc][:, b * BLK:b * BLK + wb],
                            start=(cc == 0), stop=(cc == CC - 1),
                            skip_group_check=sgc)
                    if cc == CC - 1:
                        for b in bs:
                            wb = min(BLK, W - b * BLK)
                            nc.vector.tensor_scalar(
                                out=u[:, b * BLK:b * BLK + wb],
                                in0=hps[b][:, :wb],
                                scalar1=cvec[:, 0:1], scalar2=0.0,
                                op0=Alu.add, op1=Alu.max)
                        if p == npair - 1:
                            # tail column: same relu path on z=0
                            nc.vector.tensor_scalar(
                                out=u[:, W:W + 1], in0=z0[:],
                                scalar1=cvec[:, 0:1],
                                scalar2=0.0, op0=Alu.add, op1=Alu.max)
                return go

            def tanh_fn():
                nc.scalar.activation(
                    out=hfin[:], in_=u[:], func=Act.Tanh,
                    bias=tvec_sb[:, 0:1], scale=svec_sb[:, 0:1])

            pair_hps = {}
            pieces = [make_piece(p, cc)
                      for p in range(npair) for cc in range(CC)]
            return pieces, tanh_fn, hfin

        def stage_attn(sl, xt, hfin, k, interject=()):
            """Per (cc, quad): W2 -> exp(+se accum) into slot-wide e tiles.
            `interject[i]` closures are emitted after the i-th (cc,q) step,
            so their DVE/PE work overlaps this slot's Act-bound exp stream.
            The prod pass is emitted separately (stage_prod).
            Returns (se3, etiles)."""
            W, nq = widths[sl], NQ[sl]
            interject = dict(interject)
            step = 0
            se3 = accp.tile([128, CC, nq], f32, tag=f"se3_{sl}",
                            name=f"se3_{sl}_{k}")
            etiles = [epool.tile([128, widths[sl] + 1], bf16, bufs=1,
                                 tag=f"e{sl}_{cc}", name=f"e{sl}_{cc}_{k}")
                      for cc in range(CC)]
            for cc in range(CC):
                for q in range(nq):
                    q0 = q * QUAD
                    wq = min(QUAD, W - q0)
                    last = (q == nq - 1)
                    wqe = wq + (1 if last else 0)   # + tail column
                    a_ps = pa.tile([128, QUAD], f32, tag="a_ps",
                                   name=f"a{sl}_{cc}_{q}_{k}")
                    if OPTS["w2wide"]:
                        nc.tensor.matmul(
                            a_ps[:, :wqe],
                            w2T_sb[:, cc * 128:(cc + 1) * 128],
                            hfin[:, q0:q0 + wqe],
                            start=True, stop=True, skip_group_check=sgc)
                    else:
                        for h0 in range(0, wqe, BLK):
                            wh = min(BLK, wqe - h0)
                            nc.tensor.matmul(
                                a_ps[:, h0:h0 + wh],
                                w2T_sb[:, cc * 128:(cc + 1) * 128],
                                hfin[:, q0 + h0:q0 + h0 + wh],
                                start=True, stop=True, skip_group_check=sgc)
                    nc.scalar.activation(
                        out=etiles[cc][:, q0:q0 + wqe], in_=a_ps[:, :wqe],
                        func=Act.Exp, accum_out=se3[:, cc, q:q + 1])
                    for fn in interject.get(step, ()):
                        fn()
                    step += 1
            return se3, etiles

        def stage_prod(sl, xt, etiles, k):
            """Slot-wide prod: per cc one TT multiply (2x) + one 4x
            accumulate; plus the tail-correction minis.
            Returns (sp2, ecorr)."""
            W = widths[sl]
            sp2 = accp.tile([128, CC], f32, tag=f"sp2_{sl}",
                            name=f"sp2_{sl}_{k}")
            ecorr = smalls.tile([128, CC], f32, tag=f"ecorr{sl}",
                                name=f"ecorr{sl}_{k}")
            for cc in range(CC):
                tmp = spool.tile([128, W0MAX], bf16, tag="ptmp",
                                 name=f"pt{sl}_{cc}_{k}")
                nc.vector.tensor_tensor(
                    out=tmp[:, :W], in0=etiles[cc][:, :W],
                    in1=xt[sl][cc][:, :W], op=Alu.mult)
                scr = spool.tile([128, W0MAX], bf16, tag="scr",
                                 name=f"p{sl}_{cc}_{k}")
                nc.vector.tensor_scalar(
                    out=scr[:, :W], in0=tmp[:, :W], scalar1=0.0,
                    scalar2=None, op0=Alu.bypass, op1=Alu.add,
                    accum_out=sp2[:, cc:cc + 1])
                # ecorr[cc] = wtail * e_tail  (appended tail column)
                nc.vector.tensor_scalar(
                    out=ecorr[:, cc:cc + 1], in0=etiles[cc][:, W:W + 1],
                    scalar1=wtail_sb[:, sl:sl + 1], scalar2=None,
                    op0=Alu.mult)
            return sp2, ecorr

        def stage_final(sl, se3, sp2, ecorr, out_sb, k):
            se_t = smalls.tile([128, CC], f32, tag="se_t", name=f"se_t{sl}_{k}")
            nc.vector.tensor_reduce(out=se_t[:], in_=se3[:],
                                    axis=mybir.AxisListType.X, op=Alu.add)
            nc.vector.tensor_tensor(out=se_t[:], in0=se_t[:], in1=ecorr[:],
                                    op=Alu.subtract)
            rec = smalls.tile([128, CC], f32, tag="rec", name=f"rec{sl}_{k}")
            nc.vector.reciprocal(out=rec[:], in_=se_t[:])
            mean_o = out_sb[:, sl * 4:sl * 4 + 4]
            nc.vector.tensor_tensor(out=mean_o, in0=sp2[:], in1=rec[:],
                                    op=Alu.mult)
            var_t = smalls.tile([128, CC], f32, tag="var_t",
                                name=f"var_t{sl}_{k}")
            nc.vector.tensor_tensor(out=var_t[:], in0=mean_o,
                                    in1=mean_o, op=Alu.mult)
            nc.vector.tensor_tensor(out=var_t[:], in0=mean_o, in1=var_t[:],
                                    op=Alu.subtract)
            nc.vector.tensor_scalar(out=var_t[:], in0=var_t[:], scalar1=EPS,
                                    scalar2=None, op0=Alu.max)
            if OPTS["act_sqrt"]:
                nc.scalar.sqrt(out=out_sb[:, 8 + sl * 4:8 + sl * 4 + 4],
                               in_=var_t[:])
            else:
                newton_sqrt(smalls, var_t, CC, OPTS["newton_iters"],
                            out=out_sb[:, 8 + sl * 4:8 + sl * 4 + 4])

        # -------------------------------------------------- emission

        def spread(pieces, nsteps):
            """Distribute closures over the attn (cc,q) steps, evenly."""
            sched = {}
            npc = len(pieces)
            for i, fn in enumerate(pieces):
                slot = min(nsteps - 1, ((i + 1) * nsteps) // (npc + 1))
                sched.setdefault(slot, []).append(fn)
            return sched

        def emit_chain(nbody):
            """Software-pipelined chain of `nbody` iterations. Next body's
            loads/sum-x/stats/W1a/relu are interleaved into this body's
            Act-bound exp streams so Act never waits at body boundaries.
            Stats-finish closures ride INSIDE the following attn stream to
            avoid head-of-line blocking the PE queue with dependent matmuls."""
            cur_x = stage_load("h")
            cvs = []
            for sl in range(2):
                sxp, xs2_ = stats_sx_pieces(sl, cur_x, "h")
                for f in sxp:
                    f()
                cv, fill = stats_finish(sl, xs2_, "h")
                fill()
                cvs.append(cv)
            hp, tanh0, hf0 = stage_h_pieces(0, cur_x, cvs[0], "h")
            for f in hp:
                f()
            tanh0()

            # st carries: x views, cvec handles, slot0 hfin, and the
            # previous body's deferred finals closure
            st = {"x": cur_x, "cv": cvs, "hf0": hf0, "finals": None}
            for k in range(nbody):
                emit_next = k < nbody - 1
                new_x = stage_load(k) if emit_next else None
                # slot0 attn interject: prev finals early, h(s1,k) pieces,
                # then sx0(k+1) + stats0(k+1) fill late
                hp1, tanh1, hf1 = stage_h_pieces(1, st["x"], st["cv"][1], k)
                ns0 = CC * NQ[0]
                sched0 = {}
                if st["finals"]:
                    sched0[ns0 - 2] = [st["finals"]]
                for i, fn in enumerate(hp1):
                    sched0.setdefault(min(ns0 - 1, 2 + (i * 5) // len(hp1)),
                                      []).append(fn)
                if emit_next:
                    sxp0, xs2n0 = stats_sx_pieces(0, new_x, f"{k}n")
                    for i, fn in enumerate(sxp0):
                        sched0.setdefault(min(ns0 - 1, 7 + i), []).append(fn)
                    cvn0, fill_n0 = stats_finish(0, xs2n0, f"{k}n")
                    sched0.setdefault(ns0 - 1, []).append(fill_n0)
                se0, et0 = stage_attn(0, st["x"], st["hf0"], k, sched0)
                tanh1()
                # slot1 attn interject: h(s0,k+1) pieces, sx1(k+1) +
                # stats1(k+1) fill late
                ns1 = CC * NQ[1]
                sched1 = {}
                if emit_next:
                    hp0n, tanh0n, hf0n = stage_h_pieces(0, new_x, cvn0,
                                                        f"{k}n")
                    for i, fn in enumerate(hp0n):
                        sched1.setdefault(min(ns1 - 1, 1 + (i * 4) // len(hp0n)),
                                          []).append(fn)
                    sxp1n, xs2n1 = stats_sx_pieces(1, new_x, f"{k}n")
                    for i, fn in enumerate(sxp1n):
                        sched1.setdefault(min(ns1 - 1, 5 + (i // 2)), []).append(fn)
                    cvn1, fill_n1 = stats_finish(1, xs2n1, f"{k}n")
                    sched1.setdefault(ns1 - 1, []).append(fill_n1)
                se1, et1 = stage_attn(1, st["x"], hf1, k, sched1)
                if emit_next:
                    tanh0n()
                sp0, ec0 = stage_prod(0, st["x"], et0, k)
                sp1, ec1 = stage_prod(1, st["x"], et1, k)

                def make_finals(se0=se0, sp0=sp0, ec0=ec0, se1=se1,
                                sp1=sp1, ec1=ec1, k=k):
                    def go():
                        out_sb = outp.tile([128, 16], f32, tag="out_sb",
                                           name=f"out_{k}")
                        stage_final(0, se0, sp0, ec0, out_sb, k)
                        stage_final(1, se1, sp1, ec1, out_sb, k)
                        nc.sync.dma_start(out, out_sb[:])
                    return go

                if emit_next:
                    st = {"x": new_x, "cv": [cvn0, cvn1], "hf0": hf0n,
                          "finals": make_finals()}
                else:
                    make_finals()()

        if OPTS.get("sim_chain"):
            emit_chain(OPTS["sim_chain"])
        elif loop:
            reps_sb = consts.tile([1, 1], i32, name="reps_sb", tag="reps_sb")
            nc.sync.dma_start(reps_sb[:], reps)
            regs = nc.alloc_registers("reps_regs")
            nc.regs_load(regs, reps_sb[:1, :1])
            rv = nc.snap(regs, donate=True)
            with tc.For_i(0, rv, OPTS["unroll"],
                          staggered_reset=OPTS.get("stagger", False)):
                emit_chain(OPTS["unroll"])
        else:
            emit_chain(1)

    nc.compile()
    return nc


# ---------------------------------------------------------------- interface

_PROGRAM_CACHE = {}


def _get_program(widths, loop=False):
    key = (tuple(widths), loop, str(sorted(OPTS.items())))
    if key not in _PROGRAM_CACHE:
        _PROGRAM_CACHE[key] = _build_program(widths, loop=loop)
    return _PROGRAM_CACHE[key]


def _prepare(inputs, loop=False):
    in_maps, metas, widths = _host_prep(**inputs)
    nc = _get_program(widths, loop=loop)
    return nc, in_maps, metas


def _gather(results, metas):
    pooled = np.zeros((N, 2 * C, 1), dtype=np.float32)
    for core in range(NCORES):
        o = np.asarray(results[core]["out"])   # [128, 16]
        for sl in range(2):
            n, _T = metas[core][sl]
            pooled[n, :C, 0] = o[:, sl * 4:sl * 4 + 4].T.reshape(C)
            pooled[n, C:, 0] = o[:, 8 + sl * 4:8 + sl * 4 + 4].T.reshape(C)
    return pooled


def kernel(**inputs):
    from concourse.bass_utils import run_bass_kernel_spmd
    nc, in_maps, metas = _prepare(inputs)
    res = run_bass_kernel_spmd(nc, in_maps, core_ids=list(range(NCORES)))
    return _gather(res.results, metas)
